# revision 1
# baseline (speedup 1.0000x reference)
"""Trainium2 Bass kernel for nn_RadialModel (forward NUFFT, radial MRI).

Per-core (1 frame, all 8 coils):
  1. coil multiply (DVE)                 cimage = (xr+ixi)*(cr+ici)
  2. DFT via PE matmuls (two stages):    G[v,u] = A @ (M^T @ A^T)  with
     apodization + fftshift phases folded into the constant A matrices
  3. store grid to a DRAM table (bf16), coil-interleaved cells
     [p=v_pad(517), q=u_pad(517), cri(16)] with 2/3-cell wraparound halo
  4. Kaiser-Bessel 6x6 interpolation: indirect-DMA gathers (one 6-cell x
     16-cri 192B chunk per point per row-tap; HW allows 1 index per
     partition per call -> 768 calls), weighted reduce on DVE
  5. sqrt(w) scale + store

Sharding: one frame (nt) per NeuronCore, 8 cores. Host does only
shard/reshape/unshuffle; all math on device.
"""
import math
import numpy as np

import concourse.bass as bass
import concourse.bacc as bacc
import concourse.mybir as mybir
import concourse.tile as tile
from concourse.bass_utils import run_bass_kernel_spmd
from concourse.masks import make_identity

F32 = mybir.dt.float32
I32 = mybir.dt.int32
AX = mybir.AxisListType
OP = mybir.AluOpType

IM = 256
G = 512
J = 6
ALPHA = 2.34 * J
TWO_PI = 2.0 * np.pi
PAD = 517          # 512 + 2 left halo + 3 right halo
NT, NC, K = 8, 8, 16384
NCH = NC // 2      # coils per stage-1 half (SBUF pressure)
CELL = NC * 2      # floats per (p,q) cell = 16 (all coils interleaved)
TW = PAD * CELL    # table row width in elements = 8272
NTILE = 16         # point tiles of 1024 points (8 groups x 128 partitions)
GRP = 8            # groups per tile
DEG = 8            # KB weight polynomial degree (in t); abs err ~8.5e-6


# ---------------------------------------------------------------- host consts
def _host_consts():
    # apodization correction 1/FT(kb)
    f = (np.arange(IM) - IM // 2) / G
    z = (np.pi * J * f) ** 2 - ALPHA ** 2
    s = np.sqrt(np.abs(z))
    val = np.where(z < 0, np.sinh(s) / np.maximum(s, 1e-12), np.sinc(s / np.pi))
    ftkb = (J / np.i0(ALPHA)) * val
    scal = 1.0 / ftkb
    # A[u, x'] = e^{i pi u/2 - 2 pi i u x'/G} * scal[x'] / sqrt(G)
    u = np.arange(G)[:, None].astype(np.float64)
    xp = np.arange(IM)[None, :].astype(np.float64)
    A = np.exp(1j * np.pi * u / 2 - 2j * np.pi * u * xp / G) * scal[None, :] / np.sqrt(G)
    art = np.ascontiguousarray(A.T.real, dtype=np.float32)   # [256, 512]
    ait = np.ascontiguousarray(A.T.imag, dtype=np.float32)
    aitn = np.ascontiguousarray(-A.T.imag, dtype=np.float32)
    # polynomial fit of w(t) = i0(ALPHA*sqrt(t))/i0(ALPHA) on t in [0,1]
    n = 512
    x = (1 - np.cos(np.pi * (np.arange(n) + 0.5) / n)) / 2
    w = np.i0(ALPHA * np.sqrt(x)) / np.i0(ALPHA)
    V = np.vander(x, DEG + 1, increasing=True)
    c, *_ = np.linalg.lstsq(V, w, rcond=None)
    return art, ait, aitn, c.astype(np.float64)


_ART, _AIT, _AITN, _CHEB = _host_consts()


# ---------------------------------------------------------------- bass build
def build_bass(debug=False):
    nc = bacc.Bacc()

    x_in = nc.declare_dram_parameter("x", [2, IM, IM], F32, isOutput=False)
    k_in = nc.declare_dram_parameter("kk", [2, K], F32, isOutput=False)
    c_in = nc.declare_dram_parameter("coil", [NC, 2, IM, IM], F32, isOutput=False)
    w_in = nc.declare_dram_parameter("wr", [128, NTILE * 128], F32, isOutput=False)
    art_in = nc.declare_dram_parameter("art", [IM, G], F32, isOutput=False)
    ait_in = nc.declare_dram_parameter("ait", [IM, G], F32, isOutput=False)
    aitn_in = nc.declare_dram_parameter("aitn", [IM, G], F32, isOutput=False)
    y_out = nc.declare_dram_parameter("yr", [128, NTILE * 128], F32, isOutput=True)

    BF16 = mybir.dt.bfloat16
    T_dram = nc.dram_tensor("T0", [PAD, TW], BF16)

    CH = _CHEB
    with tile.TileContext(nc) as tc:
        with (
            tc.tile_pool(name="const", bufs=1) as constp,
            tc.tile_pool(name="work", bufs=1) as workp,
            tc.tile_pool(name="ctile", bufs=2) as coilp,
            tc.tile_pool(name="mtile", bufs=4) as mp,
            tc.tile_pool(name="bt", bufs=8) as btp,
            tc.tile_pool(name="stg", bufs=1) as stgp,
            tc.tile_pool(name="patch", bufs=2) as patchp,
            tc.tile_pool(name="w36", bufs=2) as w36p,
            tc.tile_pool(name="wp", bufs=2) as wpp,
            tc.tile_pool(name="ps1", bufs=4, space="PSUM") as ps1,
            tc.tile_pool(name="ps2", bufs=4, space="PSUM") as ps2,
        ):
            # ---------------- constants ----------------
            ident = constp.tile([128, 128], F32, tag="ident")
            make_identity(nc, ident[:])
            art = []
            for name, src in (("art", art_in), ("ait", ait_in), ("aitn", aitn_in)):
                ts_ = []
                for xt in range(2):
                    t_ = constp.tile([128, G], F32, tag=f"{name}{xt}")
                    nc.sync.dma_start(out=t_[:], in_=src[xt * 128:(xt + 1) * 128, :])
                    ts_.append(t_)
                art.append(ts_)
            artT, aitT, aitnT = art

            offs = constp.tile([128, J], F32, tag="offs")
            cbt = constp.tile([128, J], F32, tag="cbt")
            for a in range(J):
                nc.vector.memset(offs[:, a:a + 1], float(3 - (a + 1)))
                nc.vector.memset(cbt[:, a:a + 1], float(((a + 1) + 2) * PAD + 3))

            # ---------------- k -> [p, c] transpose ----------------
            kg = workp.tile([128, 256], F32, tag="kg")  # [p, (d, c)]
            for d in range(2):
                kt_in = workp.tile([128, 128], F32, tag="ktin")
                nc.sync.dma_start(
                    out=kt_in[:], in_=k_in[d].rearrange("(c p) -> c p", p=128)
                )
                ktp = ps2.tile([128, 128], F32, tag="psb")
                nc.tensor.transpose(ktp[:], kt_in[:], ident[:])
                nc.scalar.copy(out=kg[:, d * 128:(d + 1) * 128], in_=ktp[:])

            # ---------------- w load + sqrt ----------------
            wsq = workp.tile([128, NTILE * 128], F32, tag="wsq")
            nc.sync.dma_start(out=wsq[:], in_=w_in[:])
            nc.scalar.activation(
                out=wsq[:], in_=wsq[:],
                func=mybir.ActivationFunctionType.Sqrt,
            )

            # ---------------- index & weight math (DVE) ----------------
            # gx = om*(G/2pi); gx += 512 if gx < 0  -> [0, 512)
            gx0 = workp.tile([128, 256], F32, tag="gx0")
            nc.vector.tensor_scalar_mul(gx0[:], kg[:], float(G / TWO_PI))
            msk = workp.tile([128, 256], F32, tag="msk")
            nc.vector.tensor_scalar(
                out=msk[:], in0=gx0[:], scalar1=0.0, scalar2=None, op0=OP.is_lt
            )
            gxy = workp.tile([128, 256], F32, tag="gxy")
            nc.vector.scalar_tensor_tensor(
                out=gxy[:], in0=msk[:], scalar=float(G), in1=gx0[:],
                op0=OP.mult, op1=OP.add,
            )
            # gm3 = gxy - 3 ; f = rne(gm3 - 0.498) via 2^23 trick ; r = gm3 - f
            gm3 = workp.tile([128, 256], F32, tag="gm3")
            nc.vector.tensor_scalar(
                out=gm3[:], in0=gxy[:], scalar1=3.0, scalar2=None, op0=OP.subtract
            )
            fl = workp.tile([128, 256], F32, tag="fl")
            nc.vector.tensor_scalar(
                out=fl[:], in0=gm3[:],
                scalar1=-0.498046875, scalar2=12582912.0,
                op0=OP.add, op1=OP.add,
            )
            nc.vector.tensor_scalar(
                out=fl[:], in0=fl[:], scalar1=12582912.0, scalar2=None,
                op0=OP.subtract,
            )
            rr = workp.tile([128, 256], F32, tag="rr")
            nc.vector.tensor_sub(rr[:], gm3[:], fl[:])

            # U[p, (dc, a)] = r + (3 - a_idx)
            ut = workp.tile([128, 256 * J], F32, tag="ut")
            ut3 = ut[:].rearrange("p (dc a) -> p dc a", a=J)
            nc.vector.tensor_tensor(
                out=ut3,
                in0=rr[:].unsqueeze(2).broadcast_to([128, 256, J]),
                in1=offs[:].unsqueeze(1).broadcast_to([128, 256, J]),
                op=OP.add,
            )
            # t = max(0, 1 - (U/3)^2)
            tsq = workp.tile([128, 256 * J], F32, tag="tsq")
            nc.vector.tensor_mul(tsq[:], ut[:], ut[:])
            nc.vector.tensor_scalar(
                out=tsq[:], in0=tsq[:], scalar1=float(-1.0 / 9.0), scalar2=1.0,
                op0=OP.mult, op1=OP.add,
            )
            nc.vector.tensor_scalar_max(tsq[:], tsq[:], 0.0)
            # Horner in t
            acc = workp.tile([128, 256 * J], F32, tag="acc")
            nc.vector.tensor_scalar(
                out=acc[:], in0=tsq[:], scalar1=float(CH[DEG]),
                scalar2=float(CH[DEG - 1]), op0=OP.mult, op1=OP.add,
            )
            for d in range(DEG - 2, -1, -1):
                nc.vector.tensor_mul(acc[:], acc[:], tsq[:])
                nc.vector.tensor_scalar_add(acc[:], acc[:], float(CH[d]))
            # acc = W_all [p, (d, c, a)]: d=0 -> wx taps, d=1 -> wy taps

            # gather cell indices: flat = fy*517 + (b+2)*517 + 3 + fx
            fy517 = workp.tile([128, 128], F32, tag="fy517")
            nc.vector.tensor_scalar_mul(fy517[:], fl[:, 128:256], float(PAD))
            idxf = workp.tile([128, 128 * J], F32, tag="idxf")
            idxf3 = idxf[:].rearrange("p (c b) -> p c b", b=J)
            nc.vector.tensor_tensor(
                out=idxf3,
                in0=fy517[:].unsqueeze(2).broadcast_to([128, 128, J]),
                in1=cbt[:].unsqueeze(1).broadcast_to([128, 128, J]),
                op=OP.add,
            )
            nc.vector.tensor_tensor(
                out=idxf3,
                in0=idxf3,
                in1=fl[:, 0:128].unsqueeze(2).broadcast_to([128, 128, J]),
                op=OP.add,
            )
            idx32 = workp.tile([128, 128 * J], I32, tag="idx32")
            nc.vector.tensor_copy(out=idx32[:], in_=idxf[:])

            # ---------------- res buffer ----------------
            res = workp.tile([128, NTILE * 128], F32, tag="res")

            # x image tiles (persist across all coils)
            xts = []
            for xt in range(2):
                xt_t = workp.tile([128, 2 * IM], F32, tag=f"xt{xt}")
                nc.sync.dma_start(
                    out=xt_t[:],
                    in_=x_in[:, xt * 128:(xt + 1) * 128, :]
                    .rearrange("ri x y -> x ri y"),
                )
                xts.append(xt_t)

            # 4 persistent bf16 stagings (one per v-tile), filled across coils
            stgs = []
            for vt in range(4):
                stg = stgp.tile([128, G * CELL], BF16, tag=f"stg{vt}")
                stgs.append(stg)

            for c in range(NC):
                # ---- coil multiply ----
                mt = []
                for xt in range(2):
                    ct = coilp.tile([128, 2 * IM], F32, tag="ct")
                    nc.sync.dma_start(
                        out=ct[:],
                        in_=c_in[c, :, xt * 128:(xt + 1) * 128, :]
                        .rearrange("ri x y -> x ri y"),
                    )
                    xt_t = xts[xt]
                    m = mp.tile([128, 2 * IM], F32, tag="m")
                    xr, xi = xt_t[:, 0:IM], xt_t[:, IM:2 * IM]
                    cr, ci = ct[:, 0:IM], ct[:, IM:2 * IM]
                    mr, mi = m[:, 0:IM], m[:, IM:2 * IM]
                    t1 = mp.tile([128, IM], F32, tag="cm1")
                    t2 = mp.tile([128, IM], F32, tag="cm2")
                    nc.vector.tensor_mul(t1[:], xr, cr)
                    nc.vector.tensor_mul(t2[:], xi, ci)
                    nc.vector.tensor_sub(mr, t1[:], t2[:])
                    nc.vector.tensor_mul(t1[:], xr, ci)
                    nc.vector.tensor_mul(t2[:], xi, cr)
                    nc.vector.tensor_add(mi, t1[:], t2[:])
                    mt.append(m)
                # ---- stage 1: BT[y, u] per (ri, Yt) ----
                bt = {}
                for yt in range(2):
                    pr = ps1.tile([128, G], F32, tag="psa")
                    pi = ps1.tile([128, G], F32, tag="psa")
                    for xt in range(2):
                        mrb = mt[xt][:, yt * 128:yt * 128 + 128]
                        mib = mt[xt][:, IM + yt * 128:IM + yt * 128 + 128]
                        st = xt == 0
                        sp = xt == 1
                        nc.tensor.matmul(pr[:], mrb, artT[xt][:], start=st, stop=False)
                        nc.tensor.matmul(pi[:], mrb, aitT[xt][:], start=st, stop=False)
                        nc.tensor.matmul(pr[:], mib, aitnT[xt][:], start=False, stop=sp)
                        nc.tensor.matmul(pi[:], mib, artT[xt][:], start=False, stop=sp)
                    btr = btp.tile([128, G], F32, tag="bt")
                    bti = btp.tile([128, G], F32, tag="bt")
                    nc.scalar.copy(out=btr[:], in_=pr[:])
                    nc.scalar.copy(out=bti[:], in_=pi[:])
                    bt[(0, yt)] = btr
                    bt[(1, yt)] = bti
                # ---- stage 2: G[v, u], drain into stagings at cri slot ----
                for vt in range(4):
                    stg3 = stgs[vt][:].rearrange("p (u e) -> p u e", e=CELL)
                    gr = ps2.tile([128, G], F32, tag="psb")
                    gi = ps2.tile([128, G], F32, tag="psb")
                    for yt in range(2):
                        av = artT[yt][:, vt * 128:(vt + 1) * 128]
                        aiv = aitT[yt][:, vt * 128:(vt + 1) * 128]
                        ainv = aitnT[yt][:, vt * 128:(vt + 1) * 128]
                        btr = bt[(0, yt)]
                        bti = bt[(1, yt)]
                        st = yt == 0
                        sp = yt == 1
                        nc.tensor.matmul(gr[:], av, btr[:], start=st, stop=False)
                        nc.tensor.matmul(gi[:], aiv, btr[:], start=st, stop=False)
                        nc.tensor.matmul(gr[:], ainv, bti[:], start=False, stop=sp)
                        nc.tensor.matmul(gi[:], av, bti[:], start=False, stop=sp)
                    nc.scalar.copy(out=stg3[:, :, 2 * c:2 * c + 1], in_=gr[:].unsqueeze(2))
                    nc.scalar.copy(out=stg3[:, :, 2 * c + 1:2 * c + 2], in_=gi[:].unsqueeze(2))

            # ---- table stores: main + q halos (+ p halos at vt 0 / 3) ----
            t_stores = []
            for vt in range(4):
                stg = stgs[vt]
                Th = T_dram
                r0 = vt * 128 + 2
                t_stores.append(nc.sync.dma_start(
                    out=Th[r0:r0 + 128, 2 * CELL:2 * CELL + G * CELL], in_=stg[:]
                ))
                t_stores.append(nc.sync.dma_start(
                    out=Th[r0:r0 + 128, 514 * CELL:514 * CELL + 3 * CELL],
                    in_=stg[:, 0:3 * CELL],
                ))
                t_stores.append(nc.sync.dma_start(
                    out=Th[r0:r0 + 128, 0:2 * CELL],
                    in_=stg[:, 510 * CELL:512 * CELL],
                ))
                if vt == 0:
                    t_stores += [
                        nc.sync.dma_start(
                            out=Th[514:517, 2 * CELL:2 * CELL + G * CELL],
                            in_=stg[0:3, :],
                        ),
                        nc.sync.dma_start(
                            out=Th[514:517, 514 * CELL:514 * CELL + 3 * CELL],
                            in_=stg[0:3, 0:3 * CELL],
                        ),
                        nc.sync.dma_start(
                            out=Th[514:517, 0:2 * CELL],
                            in_=stg[0:3, 510 * CELL:512 * CELL],
                        ),
                    ]
                if vt == 3:
                    t_stores += [
                        nc.sync.dma_start(
                            out=Th[0:2, 2 * CELL:2 * CELL + G * CELL],
                            in_=stg[126:128, :],
                        ),
                        nc.sync.dma_start(
                            out=Th[0:2, 514 * CELL:514 * CELL + 3 * CELL],
                            in_=stg[126:128, 0:3 * CELL],
                        ),
                        nc.sync.dma_start(
                            out=Th[0:2, 0:2 * CELL],
                            in_=stg[126:128, 510 * CELL:512 * CELL],
                        ),
                    ]

            # ======== gather + combine ========
            tab_flat = T_dram[:].rearrange("r (q e) -> (r q) e", e=CELL)
            all_gathers = []
            for t in range(NTILE):
                w36 = w36p.tile([128, GRP * J * J], F32, tag="w36")
                w363 = w36[:].rearrange("p (g b a) -> p g b a", b=J, a=J)
                wys = acc[:, 768 + t * 48: 768 + (t + 1) * 48].rearrange(
                    "p (g b) -> p g b", b=J)
                wxs = acc[:, t * 48:(t + 1) * 48].rearrange(
                    "p (g a) -> p g a", a=J)
                nc.vector.tensor_tensor(
                    out=w363,
                    in0=wys.unsqueeze(3).broadcast_to([128, GRP, J, J]),
                    in1=wxs.unsqueeze(2).broadcast_to([128, GRP, J, J]),
                    op=OP.mult,
                )
                patch = patchp.tile([128, GRP * J * J * CELL], BF16, tag="patch")
                for g in range(GRP):
                    for b in range(J):
                        col = (t * GRP + g) * J + b
                        gi_ = nc.gpsimd.indirect_dma_start(
                            out=patch[:, (g * J + b) * J * CELL:
                                      (g * J + b + 1) * J * CELL],
                            out_offset=None,
                            in_=tab_flat,
                            in_offset=bass.IndirectOffsetOnAxis(
                                ap=idx32[:, col:col + 1], axis=0
                            ),
                        )
                        all_gathers.append(gi_)
                # WP[p, (g, cr, ba)] = patch[p, (g, b, a, cr)] * W36
                wp = wpp.tile([128, GRP * J * J * CELL], BF16, tag="wpt")
                pv = bass.AP(
                    patch[:].tensor, patch[:].offset,
                    [patch[:].ap[0],
                     [J * J * CELL, GRP], [1, CELL], [CELL, J * J]],
                )
                wv = bass.AP(
                    w36[:].tensor, w36[:].offset,
                    [w36[:].ap[0], [J * J, GRP], [0, CELL], [1, J * J]],
                )
                ov = bass.AP(
                    wp[:].tensor, wp[:].offset,
                    [wp[:].ap[0],
                     [J * J * CELL, GRP], [J * J, CELL], [1, J * J]],
                )
                nc.vector.tensor_tensor(out=ov, in0=pv, in1=wv, op=OP.mult)
                # reduce innermost (b,a)=36 -> res[:, t*128 + g*16 + cr]
                rv = bass.AP(
                    res[:].tensor, res[:].offset + t * 128,
                    [res[:].ap[0], [16, GRP], [1, CELL]],
                )
                wp3 = wp[:].rearrange("p (g cr ba) -> p g cr ba", cr=CELL, ba=J * J)
                nc.vector.tensor_reduce(out=rv, in_=wp3, axis=AX.X, op=OP.add)

            # explicit RAW edges: gathers after table stores
            for gi_ in all_gathers:
                for si in t_stores:
                    tile.add_dep_helper(gi_.ins, si.ins, reason="T RAW")

            # ======== sqrt(w) scale + store ========
            nc.vector.tensor_mul(res[:], res[:], wsq[:])
            nc.sync.dma_start(out=y_out[:], in_=res[:])

            if debug:
                dbg_outs = {
                    "kgo": kg, "acco": acc, "idxo": idx32, "flo": fl, "rro": rr,
                }
                for nm, t_ in dbg_outs.items():
                    o = nc.dram_tensor(nm, list(t_[:].shape), t_[:].dtype,
                                       kind="ExternalOutput")
                    nc.sync.dma_start(out=o[:], in_=t_[:])
                o = nc.dram_tensor("t0o", [PAD, TW], BF16, kind="ExternalOutput")
                di = nc.sync.dma_start(out=o[:], in_=T_dram[:])
                for si in t_stores:
                    tile.add_dep_helper(di.ins, si.ins, reason="T dump RAW")

    nc.compile()
    return nc


_NC_CACHE = None


def _get_nc():
    global _NC_CACHE
    if _NC_CACHE is None:
        _NC_CACHE = build_bass()
    return _NC_CACHE


# ---------------------------------------------------------------- host glue
def _shuffle_w(w_t):
    # w[c, ri, K] -> [p, (t, g, c, ri)] with K = t*1024 + g*128 + p
    v = w_t.reshape(NC, 2, NTILE, GRP, 128)
    return np.ascontiguousarray(v.transpose(4, 2, 3, 0, 1).reshape(128, NTILE * 128))


def _unshuffle_y(yr):
    # [p, (t, g, c, ri)] -> y[c, ri, K]
    v = yr.reshape(128, NTILE, GRP, NC, 2)
    return np.ascontiguousarray(v.transpose(3, 4, 1, 2, 0).reshape(NC, 2, K))


def make_in_maps(x, k, coil_sensitivities, w):
    in_maps = []
    coil0 = np.ascontiguousarray(coil_sensitivities[0], dtype=np.float32)
    for t in range(NT):
        in_maps.append({
            "x": np.ascontiguousarray(x[t], dtype=np.float32),
            "kk": np.ascontiguousarray(k[t], dtype=np.float32),
            "coil": coil0,
            "wr": _shuffle_w(np.asarray(w[t], dtype=np.float32)),
            "art": _ART, "ait": _AIT, "aitn": _AITN,
        })
    return in_maps


def run(x, k, coil_sensitivities, w, trace=False, **spmd_kwargs):
    nc = _get_nc()
    in_maps = make_in_maps(x, k, coil_sensitivities, w)
    r = run_bass_kernel_spmd(nc, in_maps, list(range(NT)), trace=trace, **spmd_kwargs)
    y = np.stack([_unshuffle_y(r.results[t]["yr"]) for t in range(NT)], axis=0)
    return y.astype(np.float32), r


def kernel(x, k, coil_sensitivities, w):
    y, _ = run(x, k, coil_sensitivities, w, trace=False)
    return y



# revision 3
# speedup vs baseline: 2.5606x; 2.5606x over previous
"""Trainium2 Bass kernel for nn_RadialModel (forward NUFFT, radial MRI).

Per-core (1 frame, all 8 coils):
  1. coil multiply (DVE, bf16 out)       cimage = (xr+ixi)*(cr+ici)
  2. DFT via PE bf16 matmuls (2 stages): G[v,u] = A @ (M^T @ A^T) with
     apodization + fftshift phases folded into the constant A matrices;
     accumulated per v-tile into stg[vt][v, u, cri(16)] bf16
  3. y-tap-interleaved table build: T2[r, q, s(5), cri(16)] bf16 in DRAM,
     T2[r,q,s] = G[(r+s-2) mod 512, (q-2) mod 512]. The s-shifts are done
     with PE matmuls against shifted-identity matrices (main + edge part
     from the neighbouring v-tile), drained straight into interleaved
     row-chunk tiles, then written with large contiguous DMAs.
  4. interpolation: 5x5 tap window centred on round(g) (the dropped 6th
     reference tap has weight < 2.5e-3).  ONE indirect-DMA descriptor per
     point fetches the whole 5-cell x 80-element 800B patch (the SWDGE
     ucode supports exactly one index per partition per call -> 128 calls
     instead of 768).  Weighted reduce on DVE.
  5. sqrt(w) scale + store

Sharding: one frame (nt) per NeuronCore, 8 cores. Host does only
shard/reshape/unshuffle; all math on device.
"""
import numpy as np

import concourse.bass as bass
import concourse.bacc as bacc
import concourse.mybir as mybir
import concourse.tile as tile
from concourse.bass_utils import run_bass_kernel_spmd
from concourse.masks import make_identity

F32 = mybir.dt.float32
I32 = mybir.dt.int32
BF16 = mybir.dt.bfloat16
AX = mybir.AxisListType
OP = mybir.AluOpType

IM = 256
G = 512
J = 6              # Kaiser-Bessel width of the reference kernel
ALPHA = 2.34 * J
TWO_PI = 2.0 * np.pi
NT, NC, K = 8, 8, 16384
TAPS = 5           # tap window per dim (centre 5 of the 6 reference taps)
CRI = NC * 2       # 16 interleaved coil-re/im values
CELL = TAPS * CRI  # 80 elements per (r, q) table cell
QC = 517           # table cells per row: 2 left halo + 512 + 3 right
ROWS = G
NTILE = 16         # point tiles of 1024 points (8 groups x 128 partitions)
GRP = 8
DEG = 8            # KB weight polynomial degree; abs err ~8.5e-6


# ---------------------------------------------------------------- host consts
def _host_consts():
    # apodization correction 1/FT(kb)
    f = (np.arange(IM) - IM // 2) / G
    z = (np.pi * J * f) ** 2 - ALPHA ** 2
    s = np.sqrt(np.abs(z))
    val = np.where(z < 0, np.sinh(s) / np.maximum(s, 1e-12), np.sinc(s / np.pi))
    ftkb = (J / np.i0(ALPHA)) * val
    scal = 1.0 / ftkb
    # A[u, x'] = e^{i pi u/2 - 2 pi i u x'/G} * scal[x'] / sqrt(G)
    u = np.arange(G)[:, None].astype(np.float64)
    xp = np.arange(IM)[None, :].astype(np.float64)
    A = np.exp(1j * np.pi * u / 2 - 2j * np.pi * u * xp / G) * scal[None, :] / np.sqrt(G)
    art = np.ascontiguousarray(A.T.real, dtype=np.float32)   # [256, 512]
    ait = np.ascontiguousarray(A.T.imag, dtype=np.float32)
    aitn = np.ascontiguousarray(-A.T.imag, dtype=np.float32)
    # polynomial fit of w(t) = i0(ALPHA*sqrt(t))/i0(ALPHA) on t in [0,1]
    n = 512
    x = (1 - np.cos(np.pi * (np.arange(n) + 0.5) / n)) / 2
    w = np.i0(ALPHA * np.sqrt(x)) / np.i0(ALPHA)
    V = np.vander(x, DEG + 1, increasing=True)
    c, *_ = np.linalg.lstsq(V, w, rcond=None)
    # shifted identities [v, r]: main M_d (v = r+d) and edge E_d from the
    # neighbouring v-tile, for d in (-2, -1, 1, 2) -> slots (2i, 2i+1)
    sh = np.zeros((8, 128, 128), np.float32)
    for i, d in enumerate((-2, -1, 1, 2)):
        sh[2 * i] = np.eye(128, k=-d, dtype=np.float32)
        sh[2 * i + 1] = np.eye(128, k=(128 - d) if d > 0 else (-128 - d),
                               dtype=np.float32)
    return art, ait, aitn, c.astype(np.float64), \
        np.ascontiguousarray(sh.reshape(8 * 128, 128))


_ART, _AIT, _AITN, _CHEB, _SHIFTS = _host_consts()


# ---------------------------------------------------------------- bass build
def build_bass(debug=False):
    nc = bacc.Bacc()

    x_in = nc.declare_dram_parameter("x", [2, IM, IM], F32, isOutput=False)
    k_in = nc.declare_dram_parameter("kk", [2, K], F32, isOutput=False)
    c_in = nc.declare_dram_parameter("coil", [NC, 2, IM, IM], F32, isOutput=False)
    w_in = nc.declare_dram_parameter("wr", [128, NTILE * 128], F32, isOutput=False)
    art_in = nc.declare_dram_parameter("art", [IM, G], F32, isOutput=False)
    ait_in = nc.declare_dram_parameter("ait", [IM, G], F32, isOutput=False)
    aitn_in = nc.declare_dram_parameter("aitn", [IM, G], F32, isOutput=False)
    sh_in = nc.declare_dram_parameter("shifts", [8 * 128, 128], F32, isOutput=False)
    y_out = nc.declare_dram_parameter("yr", [128, NTILE * 128], F32, isOutput=True)

    T2 = nc.dram_tensor("T2", [ROWS, QC * CELL], BF16)

    CH = _CHEB
    with tile.TileContext(nc) as tc:
        with (
            tc.tile_pool(name="const", bufs=1) as constp,
            tc.tile_pool(name="work", bufs=1) as workp,
            tc.tile_pool(name="ctile", bufs=2) as coilp,
            tc.tile_pool(name="mtile", bufs=4) as mp,
            tc.tile_pool(name="bt", bufs=6) as btp,
            tc.tile_pool(name="stg", bufs=1) as stgp,
            tc.tile_pool(name="rowc", bufs=2) as rowcp,
            tc.tile_pool(name="patch", bufs=2) as patchp,
            tc.tile_pool(name="w25", bufs=2) as w25p,
            tc.tile_pool(name="wp", bufs=2) as wpp,
            tc.tile_pool(name="ps1", bufs=4, space="PSUM") as ps1,
            tc.tile_pool(name="ps2", bufs=4, space="PSUM") as ps2,
        ):
            # ---------------- constants ----------------
            ident = constp.tile([128, 128], F32, tag="ident")
            make_identity(nc, ident[:])
            fstg = workp.tile([128, G], F32, tag="fstg")
            art = []
            for name, src in (("art", art_in), ("ait", ait_in), ("aitn", aitn_in)):
                ts_ = []
                for xt in range(2):
                    nc.sync.dma_start(
                        out=fstg[:], in_=src[xt * 128:(xt + 1) * 128, :])
                    t_ = constp.tile([128, G], BF16, tag=f"{name}{xt}")
                    nc.vector.tensor_copy(out=t_[:], in_=fstg[:])
                    ts_.append(t_)
                art.append(ts_)
            artT, aitT, aitnT = art

            # shifted identities (bf16), slots: d=(-2,-1,1,2) -> (main, edge)
            shm = {}
            she = {}
            for i, d in enumerate((-2, -1, 1, 2)):
                nc.sync.dma_start(
                    out=fstg[:, 0:128],
                    in_=sh_in[(2 * i) * 128:(2 * i + 1) * 128, :])
                tm = constp.tile([128, 128], BF16, tag=f"shm{i}")
                nc.vector.tensor_copy(out=tm[:], in_=fstg[:, 0:128])
                shm[d] = tm
                nc.sync.dma_start(
                    out=fstg[:, 0:128],
                    in_=sh_in[(2 * i + 1) * 128:(2 * i + 2) * 128, :])
                te = constp.tile([128, 128], BF16, tag=f"she{i}")
                nc.vector.tensor_copy(out=te[:], in_=fstg[:, 0:128])
                she[d] = te

            offs = constp.tile([128, TAPS], F32, tag="offs")
            for a in range(TAPS):
                nc.vector.memset(offs[:, a:a + 1], float(2 - a))

            # ---------------- k -> [p, c] transpose ----------------
            kg = workp.tile([128, 256], F32, tag="kg")  # [p, (d, c)]
            for d in range(2):
                kt_in = workp.tile([128, 128], F32, tag="ktin")
                nc.sync.dma_start(
                    out=kt_in[:], in_=k_in[d].rearrange("(c p) -> c p", p=128)
                )
                ktp = ps2.tile([128, 128], F32, tag="psb")
                nc.tensor.transpose(ktp[:], kt_in[:], ident[:])
                nc.scalar.copy(out=kg[:, d * 128:(d + 1) * 128], in_=ktp[:])

            # ---------------- w load + sqrt ----------------
            wsq = workp.tile([128, NTILE * 128], F32, tag="wsq")
            nc.sync.dma_start(out=wsq[:], in_=w_in[:])
            nc.scalar.activation(
                out=wsq[:], in_=wsq[:],
                func=mybir.ActivationFunctionType.Sqrt,
            )

            # ---------------- index & weight math (DVE) ----------------
            # gxy = om*(G/2pi) mod G  -> [0, 512)
            gxy = workp.tile([128, 256], F32, tag="gxy")
            nc.vector.tensor_scalar_mul(gxy[:], kg[:], float(G / TWO_PI))
            msk = workp.tile([128, 256], F32, tag="msk")
            nc.vector.tensor_scalar(
                out=msk[:], in0=gxy[:], scalar1=0.0, scalar2=None, op0=OP.is_lt
            )
            nc.vector.scalar_tensor_tensor(
                out=gxy[:], in0=msk[:], scalar=float(G), in1=gxy[:],
                op0=OP.mult, op1=OP.add,
            )
            # fl = rne(gxy) via 2^23 trick ; f = gxy - fl in [-0.5, 0.5]
            fl = workp.tile([128, 256], F32, tag="fl")
            nc.vector.tensor_scalar(
                out=fl[:], in0=gxy[:], scalar1=12582912.0, scalar2=12582912.0,
                op0=OP.add, op1=OP.subtract,
            )
            ff = workp.tile([128, 256], F32, tag="ff")
            nc.vector.tensor_sub(ff[:], gxy[:], fl[:])
            # flm = fl mod 512  (fl in [0, 512])
            nc.vector.tensor_scalar(
                out=msk[:], in0=fl[:], scalar1=511.5, scalar2=None, op0=OP.is_gt
            )
            flm = workp.tile([128, 256], F32, tag="flm")
            nc.vector.scalar_tensor_tensor(
                out=flm[:], in0=msk[:], scalar=float(-G), in1=fl[:],
                op0=OP.mult, op1=OP.add,
            )

            # U[p, (dc, j)] = f + (2 - j)
            ut = workp.tile([128, 256 * TAPS], F32, tag="ut")
            ut3 = ut[:].rearrange("p (dc j) -> p dc j", j=TAPS)
            nc.vector.tensor_tensor(
                out=ut3,
                in0=ff[:].unsqueeze(2).broadcast_to([128, 256, TAPS]),
                in1=offs[:].unsqueeze(1).broadcast_to([128, 256, TAPS]),
                op=OP.add,
            )
            # t = 1 - (U/3)^2   (in-place square then affine)
            nc.vector.tensor_mul(ut[:], ut[:], ut[:])
            nc.vector.tensor_scalar(
                out=ut[:], in0=ut[:], scalar1=float(-1.0 / 9.0), scalar2=1.0,
                op0=OP.mult, op1=OP.add,
            )
            # Horner in t
            acc = workp.tile([128, 256 * TAPS], F32, tag="acc")
            nc.vector.tensor_scalar(
                out=acc[:], in0=ut[:], scalar1=float(CH[DEG]),
                scalar2=float(CH[DEG - 1]), op0=OP.mult, op1=OP.add,
            )
            for dd in range(DEG - 2, -1, -1):
                nc.vector.tensor_mul(acc[:], acc[:], ut[:])
                nc.vector.tensor_scalar_add(acc[:], acc[:], float(CH[dd]))
            # acc = [p, (d, c, j)]: d=0 -> wx taps, d=1 -> wy taps

            # gather cell index: idx = ry*517 + rx  (r = flm)
            fy517 = workp.tile([128, 128], F32, tag="fy517")
            nc.vector.tensor_scalar_mul(fy517[:], flm[:, 128:256], float(QC))
            idxf = workp.tile([128, 128], F32, tag="idxf")
            nc.vector.tensor_add(idxf[:], fy517[:], flm[:, 0:128])
            idx32 = workp.tile([128, 128], I32, tag="idx32")
            nc.vector.tensor_copy(out=idx32[:], in_=idxf[:])

            # ---------------- res buffer ----------------
            res = workp.tile([128, NTILE * 128], F32, tag="res")

            # x image tiles (persist across all coils), f32
            xts = []
            for xt in range(2):
                xt_t = workp.tile([128, 2 * IM], F32, tag=f"xt{xt}")
                nc.sync.dma_start(
                    out=xt_t[:],
                    in_=x_in[:, xt * 128:(xt + 1) * 128, :]
                    .rearrange("ri x y -> x ri y"),
                )
                xts.append(xt_t)

            # 4 persistent bf16 stagings [v, (u, cri)], filled across coils
            stgs = []
            for vt in range(4):
                stg = stgp.tile([128, G * CRI], BF16, tag=f"stg{vt}")
                stgs.append(stg)

            # =============== phase A: per-coil DFT ===============
            for c in range(NC):
                # ---- coil multiply (bf16 out) ----
                mt = []
                for xt in range(2):
                    ct = coilp.tile([128, 2 * IM], F32, tag="ct")
                    nc.sync.dma_start(
                        out=ct[:],
                        in_=c_in[c, :, xt * 128:(xt + 1) * 128, :]
                        .rearrange("ri x y -> x ri y"),
                    )
                    xt_t = xts[xt]
                    m = mp.tile([128, 2 * IM], BF16, tag="m")
                    xr, xi = xt_t[:, 0:IM], xt_t[:, IM:2 * IM]
                    cr, ci = ct[:, 0:IM], ct[:, IM:2 * IM]
                    mr, mi = m[:, 0:IM], m[:, IM:2 * IM]
                    t1 = mp.tile([128, IM], F32, tag="cm1")
                    t2 = mp.tile([128, IM], F32, tag="cm2")
                    nc.vector.tensor_mul(t1[:], xr, cr)
                    nc.vector.tensor_mul(t2[:], xi, ci)
                    nc.vector.tensor_sub(mr, t1[:], t2[:])
                    nc.vector.tensor_mul(t1[:], xr, ci)
                    nc.vector.tensor_mul(t2[:], xi, cr)
                    nc.vector.tensor_add(mi, t1[:], t2[:])
                    mt.append(m)
                # ---- stage 1: BT[y, u] per (ri, yt), bf16 ----
                bt = {}
                for yt in range(2):
                    pr = ps1.tile([128, G], F32, tag="psa")
                    pi = ps1.tile([128, G], F32, tag="psa")
                    for xt in range(2):
                        mrb = mt[xt][:, yt * 128:yt * 128 + 128]
                        mib = mt[xt][:, IM + yt * 128:IM + yt * 128 + 128]
                        st = xt == 0
                        sp = xt == 1
                        nc.tensor.matmul(pr[:], mrb, artT[xt][:], start=st, stop=False)
                        nc.tensor.matmul(pi[:], mrb, aitT[xt][:], start=st, stop=False)
                        nc.tensor.matmul(pr[:], mib, aitnT[xt][:], start=False, stop=sp)
                        nc.tensor.matmul(pi[:], mib, artT[xt][:], start=False, stop=sp)
                    btr = btp.tile([128, G], BF16, tag="bt")
                    bti = btp.tile([128, G], BF16, tag="bt")
                    nc.scalar.copy(out=btr[:], in_=pr[:])
                    nc.scalar.copy(out=bti[:], in_=pi[:])
                    bt[(0, yt)] = btr
                    bt[(1, yt)] = bti
                # ---- stage 2: G[v, u] -> stg[vt] cri slot ----
                for vt in range(4):
                    stg3 = stgs[vt][:].rearrange("p (u e) -> p u e", e=CRI)
                    gr = ps2.tile([128, G], F32, tag="psb")
                    gi = ps2.tile([128, G], F32, tag="psb")
                    for yt in range(2):
                        av = artT[yt][:, vt * 128:(vt + 1) * 128]
                        aiv = aitT[yt][:, vt * 128:(vt + 1) * 128]
                        ainv = aitnT[yt][:, vt * 128:(vt + 1) * 128]
                        btr = bt[(0, yt)]
                        bti = bt[(1, yt)]
                        st = yt == 0
                        sp = yt == 1
                        nc.tensor.matmul(gr[:], av, btr[:], start=st, stop=False)
                        nc.tensor.matmul(gi[:], aiv, btr[:], start=st, stop=False)
                        nc.tensor.matmul(gr[:], ainv, bti[:], start=False, stop=sp)
                        nc.tensor.matmul(gi[:], av, bti[:], start=False, stop=sp)
                    nc.scalar.copy(out=stg3[:, :, 2 * c:2 * c + 1], in_=gr[:].unsqueeze(2))
                    nc.scalar.copy(out=stg3[:, :, 2 * c + 1:2 * c + 2], in_=gi[:].unsqueeze(2))

            # =============== phase B: shift + interleave + store ===============
            t_stores = []
            drain_flip = [0]

            def _drain(out_ap, in_ap):
                # alternate drains across scalar/vector to balance engines
                if drain_flip[0] % 2 == 0:
                    nc.scalar.copy(out=out_ap, in_=in_ap)
                else:
                    nc.vector.tensor_copy(out=out_ap, in_=in_ap)
                drain_flip[0] += 1

            for vt in range(4):
                halo = workp.tile([128, 4 * CELL], BF16, tag=f"halo{vt}")
                halo4 = halo[:].rearrange("p (c s e) -> p c s e", s=TAPS, e=CRI)
                for h in range(4):
                    rc = rowcp.tile([128, 128 * CELL], BF16, tag="rc")
                    rc3 = rc[:].rearrange("p (q e) -> p q e", e=CELL)
                    for s in range(TAPS):
                        d = s - 2
                        for sub in range(4):
                            u0 = h * 128 + sub * 32
                            src = stgs[vt][:, u0 * CRI:(u0 + 32) * CRI]
                            if d == 0:
                                sview = src.rearrange("p (q e) -> p q e", e=CRI)
                                _drain(
                                    rc3[:, sub * 32:(sub + 1) * 32,
                                        s * CRI:(s + 1) * CRI],
                                    sview,
                                )
                                if h == 3 and sub == 3:
                                    _drain(halo4[:, 0:2, s, :], sview[:, 30:32, :])
                                if h == 0 and sub == 0:
                                    _drain(halo4[:, 2:4, s, :], sview[:, 0:2, :])
                                continue
                            ps = ps2.tile([128, 512], F32, tag="psb")
                            nb = stgs[(vt + (1 if d > 0 else -1)) % 4]
                            nsrc = nb[:, u0 * CRI:(u0 + 32) * CRI]
                            nc.tensor.matmul(ps[:], shm[d][:], src,
                                             start=True, stop=False)
                            nc.tensor.matmul(ps[:], she[d][:], nsrc,
                                             start=False, stop=True)
                            pview = ps[:].rearrange("p (q e) -> p q e", e=CRI)
                            _drain(
                                rc3[:, sub * 32:(sub + 1) * 32,
                                    s * CRI:(s + 1) * CRI],
                                pview,
                            )
                            if h == 3 and sub == 3:
                                _drain(halo4[:, 0:2, s, :], pview[:, 30:32, :])
                            if h == 0 and sub == 0:
                                _drain(halo4[:, 2:4, s, :], pview[:, 0:2, :])
                    t_stores.append(nc.sync.dma_start(
                        out=T2[vt * 128:(vt + 1) * 128,
                               (h * 128 + 2) * CELL:(h * 128 + 130) * CELL],
                        in_=rc[:],
                    ))
                # halo cells: q 0,1 <- u 510,511 ; q 514,515 <- u 0,1
                t_stores.append(nc.sync.dma_start(
                    out=T2[vt * 128:(vt + 1) * 128, 0:2 * CELL],
                    in_=halo[:, 0:2 * CELL],
                ))
                t_stores.append(nc.sync.dma_start(
                    out=T2[vt * 128:(vt + 1) * 128, 514 * CELL:516 * CELL],
                    in_=halo[:, 2 * CELL:4 * CELL],
                ))

            # =============== phase C: gather + combine ===============
            tab_flat = T2[:].rearrange("r (q e) -> (r q) e", e=CELL)
            all_gathers = []
            for t in range(NTILE):
                w25 = w25p.tile([128, GRP * TAPS * TAPS], F32, tag="w25")
                w253 = w25[:].rearrange("p (g a b) -> p g a b", a=TAPS, b=TAPS)
                wxs = acc[:, t * 40:(t + 1) * 40].rearrange(
                    "p (g a) -> p g a", a=TAPS)
                wys = acc[:, 640 + t * 40: 640 + (t + 1) * 40].rearrange(
                    "p (g b) -> p g b", b=TAPS)
                nc.vector.tensor_tensor(
                    out=w253,
                    in0=wxs.unsqueeze(3).broadcast_to([128, GRP, TAPS, TAPS]),
                    in1=wys.unsqueeze(2).broadcast_to([128, GRP, TAPS, TAPS]),
                    op=OP.mult,
                )
                w25b = w25p.tile([128, GRP * TAPS * TAPS], BF16, tag="w25b")
                nc.vector.tensor_copy(out=w25b[:], in_=w25[:])
                patch = patchp.tile([128, GRP * TAPS * CELL], BF16, tag="patch")
                for g in range(GRP):
                    col = t * GRP + g
                    gi_ = nc.gpsimd.indirect_dma_start(
                        out=patch[:, g * TAPS * CELL:(g + 1) * TAPS * CELL],
                        out_offset=None,
                        in_=tab_flat,
                        in_offset=bass.IndirectOffsetOnAxis(
                            ap=idx32[:, col:col + 1], axis=0
                        ),
                    )
                    all_gathers.append(gi_)
                # WP[p, (g, cr, ab)] = patch[p, (g, a, b, cr)] * W25
                wp = wpp.tile([128, GRP * TAPS * CELL], BF16, tag="wpt")
                pv = bass.AP(
                    patch[:].tensor, patch[:].offset,
                    [patch[:].ap[0],
                     [TAPS * CELL, GRP], [1, CRI], [CRI, TAPS * TAPS]],
                )
                wv = bass.AP(
                    w25b[:].tensor, w25b[:].offset,
                    [w25b[:].ap[0],
                     [TAPS * TAPS, GRP], [0, CRI], [1, TAPS * TAPS]],
                )
                ov = bass.AP(
                    wp[:].tensor, wp[:].offset,
                    [wp[:].ap[0],
                     [TAPS * CELL, GRP], [TAPS * TAPS, CRI], [1, TAPS * TAPS]],
                )
                nc.vector.tensor_tensor(out=ov, in0=pv, in1=wv, op=OP.mult)
                # reduce innermost (a,b)=25 -> res[:, t*128 + g*16 + cr]
                rv = bass.AP(
                    res[:].tensor, res[:].offset + t * 128,
                    [res[:].ap[0], [CRI, GRP], [1, CRI]],
                )
                wp3 = wp[:].rearrange("p (g cr ab) -> p g cr ab",
                                      cr=CRI, ab=TAPS * TAPS)
                nc.vector.tensor_reduce(out=rv, in_=wp3, axis=AX.X, op=OP.add)

            # explicit RAW edges: gathers after table stores
            for gi_ in all_gathers:
                for si in t_stores:
                    tile.add_dep_helper(gi_.ins, si.ins, reason="T2 RAW")

            # ======== sqrt(w) scale + store ========
            nc.vector.tensor_mul(res[:], res[:], wsq[:])
            nc.sync.dma_start(out=y_out[:], in_=res[:])

            if debug:
                dbg_outs = {
                    "kgo": kg, "acco": acc, "idxo": idx32, "flo": flm, "ffo": ff,
                }
                for nm, t_ in dbg_outs.items():
                    o = nc.dram_tensor(nm, list(t_[:].shape), t_[:].dtype,
                                       kind="ExternalOutput")
                    nc.sync.dma_start(out=o[:], in_=t_[:])
                o = nc.dram_tensor("t2o", [ROWS, QC * CELL], BF16,
                                   kind="ExternalOutput")
                di = nc.sync.dma_start(out=o[:], in_=T2[:])
                for si in t_stores:
                    tile.add_dep_helper(di.ins, si.ins, reason="T2 dump RAW")

    nc.compile()
    return nc


_NC_CACHE = None


def _get_nc():
    global _NC_CACHE
    if _NC_CACHE is None:
        _NC_CACHE = build_bass()
    return _NC_CACHE


# ---------------------------------------------------------------- host glue
def _shuffle_w(w_t):
    # w[c, ri, K] -> [p, (t, g, c, ri)] with K = t*1024 + g*128 + p
    v = w_t.reshape(NC, 2, NTILE, GRP, 128)
    return np.ascontiguousarray(v.transpose(4, 2, 3, 0, 1).reshape(128, NTILE * 128))


def _unshuffle_y(yr):
    # [p, (t, g, c, ri)] -> y[c, ri, K]
    v = yr.reshape(128, NTILE, GRP, NC, 2)
    return np.ascontiguousarray(v.transpose(3, 4, 1, 2, 0).reshape(NC, 2, K))


def make_in_maps(x, k, coil_sensitivities, w):
    in_maps = []
    coil0 = np.ascontiguousarray(coil_sensitivities[0], dtype=np.float32)
    for t in range(NT):
        in_maps.append({
            "x": np.ascontiguousarray(x[t], dtype=np.float32),
            "kk": np.ascontiguousarray(k[t], dtype=np.float32),
            "coil": coil0,
            "wr": _shuffle_w(np.asarray(w[t], dtype=np.float32)),
            "art": _ART, "ait": _AIT, "aitn": _AITN, "shifts": _SHIFTS,
        })
    return in_maps


def run(x, k, coil_sensitivities, w, trace=False, **spmd_kwargs):
    nc = _get_nc()
    in_maps = make_in_maps(x, k, coil_sensitivities, w)
    r = run_bass_kernel_spmd(nc, in_maps, list(range(NT)), trace=trace, **spmd_kwargs)
    y = np.stack([_unshuffle_y(r.results[t]["yr"]) for t in range(NT)], axis=0)
    return y.astype(np.float32), r


def kernel(x, k, coil_sensitivities, w):
    y, _ = run(x, k, coil_sensitivities, w, trace=False)
    return y
